# revision 16
# baseline (speedup 1.0000x reference)
"""Trainium2 Bass kernel for CustomMultiHeadSelfAttention.

Problem shapes: B=4, N=2048, E=1024, H=16, HD=64.

Sharding (8 cores): core c -> batch b = c//2, head-group g = c%2
(heads 8g..8g+7, i.e. feature cols [512g, 512g+512) of q/k/v).
Each core:
  - projects its batch's qkv slice -> Q^T,K^T (head-pair packed, d on
    partitions) and V (natural, seq on partitions)
  - full attention for its 8 heads (exact softmax, no max subtraction --
    scores are ~N(0,1) so exp never overflows)
  - partial out_proj: attnout_local [2048,512] @ W_out[:,cols]^T -> [2048,1024]
Host unshards: y[b] = partial[2b] + partial[2b+1] + out_proj_b.

All matmuls run in bf16 with fp32 PSUM accumulation (host pre-casts the
sharded operands); softmax statistics stay fp32.

PE utilization tricks:
  - S^T: two heads as concurrent 64-row PE tiles (row groups 0-1 / 2-3)
  - PV:  two heads as concurrent 64-col PE tiles, outputs stacked into the
    [128 = 2*64] partition layout the out-projection wants
  - softmax row-sums: ones[128,64] matmul -> sums pre-broadcast across 64
    partitions, accumulated in PSUM alongside PV

DMA: all DRAM layouts are partition-major ([128, free]) so every transfer
is long contiguous runs per partition -- descriptor-dispatch on the
sequencers was the original bottleneck.
"""

import sys

if "/opt/trn_rl_repo" not in sys.path:
    sys.path.insert(0, "/opt/trn_rl_repo")

from contextlib import ExitStack

import ml_dtypes
import numpy as np

import concourse.tile as tile
from concourse import bacc, mybir
from concourse.bass_utils import run_bass_kernel_spmd

B, N, E, H = 4, 2048, 1024, 16
HD = E // H          # 64
HL = H // 2          # 8 local heads per core
EL = HL * HD         # 512 local feature cols per core
NP = 128             # partitions
NPAIRS = HL // 2     # 4 head pairs per core (2 heads packed in 128 partitions)
QC = 512             # query chunk (free dim of S^T / PV matmuls)
NQC = N // QC        # 4
NKB = N // NP        # 16 key blocks of 128
TC = 512             # token chunk in projections
EC = E // NP         # 8 contraction chunks in the in-projection

BF16 = mybir.dt.bfloat16
FP32 = mybir.dt.float32

_CACHED = {}


def build_kernel():
    """Build the per-core Bass program (identical for every core)."""
    nc = bacc.Bacc("TRN2", target_bir_lowering=False, debug=False, num_devices=8)

    # bf16 operand blobs + one fp32 bias blob, all partition-major
    xt_d = nc.dram_tensor("xt", [NP, EC * N], BF16, kind="ExternalInput").ap()
    wt_d = nc.dram_tensor("wt", [NP, EC * 3 * EL], BF16, kind="ExternalInput").ap()
    wot_d = nc.dram_tensor("wot", [NP, NPAIRS * E], BF16, kind="ExternalInput").ap()
    bias_d = nc.dram_tensor(
        "bias", [NP, 2 * NPAIRS + EL], FP32, kind="ExternalInput"
    ).ap()
    # output, partition-major: y_d[p, tb, j] = y[tb*128 + p, j]
    y_d = nc.dram_tensor("y", [NP, NKB, E], FP32, kind="ExternalOutput").ap()

    with tile.TileContext(nc) as tc:
        _emit(tc, xt_d, wt_d, wot_d, bias_d, y_d)
    nc.compile()
    return nc


def _emit(tc, xt_d, wt_d, wot_d, bias_d, y_d):
    nc = tc.nc
    ctx = ExitStack()
    with ctx:
        singles = ctx.enter_context(tc.tile_pool(name="singles", bufs=1))
        proj_ps = ctx.enter_context(tc.tile_pool(name="proj_ps", bufs=2, space="PSUM"))
        s_ps = ctx.enter_context(tc.tile_pool(name="s_ps", bufs=2, space="PSUM"))
        pv_ps = ctx.enter_context(tc.tile_pool(name="pv_ps", bufs=1, space="PSUM"))
        pv2_ps = ctx.enter_context(tc.tile_pool(name="pv2_ps", bufs=1, space="PSUM"))
        epool = ctx.enter_context(tc.tile_pool(name="epool", bufs=4))
        rpool = ctx.enter_context(tc.tile_pool(name="rpool", bufs=2))
        dpool = ctx.enter_context(tc.tile_pool(name="dpool", bufs=2, space="DRAM"))
        ypool = ctx.enter_context(tc.tile_pool(name="ypool", bufs=1))

        # ---- resident SBUF tensors -----------------------------------------
        xt_sb = singles.tile([NP, EC, N], BF16)           # X^T  [e, t]
        wt_sb = singles.tile([NP, EC, 3 * EL], BF16)      # W_in^T [e, c]
        wot_sb = singles.tile([NP, NPAIRS, E], BF16)      # W_out^T [el, j]
        qt_sb = singles.tile([NP, NPAIRS, N], BF16)       # Q^T (pair-packed)
        kt_sb = singles.tile([NP, NPAIRS, N], BF16)       # K^T (pair-packed)
        # V packed per head as a 128-col stationary block: even head in a
        # pair -> [V_h | ones], odd head -> [ones | V_h].  The ones block
        # makes every PV matmul also produce that head's softmax row-sums,
        # broadcast across 64 partitions, on the half not holding data.
        vo_sb = singles.tile([NP, NKB, HL, NP], BF16)
        at_sb = singles.tile([NP, NPAIRS, N], BF16)       # attnout^T (pair-packed)
        bias_sb = singles.tile([NP, 2 * NPAIRS + EL], FP32)

        xt_dv = xt_d.rearrange("p (ec t) -> p ec t", ec=EC)
        wt_dv = wt_d.rearrange("p (ec c) -> p ec c", ec=EC)
        # V-weights first, then Q/K weights; X^T in quarters -- so the first
        # V-projection matmul isn't gated on the whole 8.4 MB input load
        nc.sync.dma_start(wt_sb[:, :, 2 * EL:3 * EL], wt_dv[:, :, 2 * EL:3 * EL])
        nc.sync.dma_start(xt_sb[:, :, 0:N // 4], xt_dv[:, :, 0:N // 4])
        nc.sync.dma_start(wt_sb[:, :, 0:2 * EL], wt_dv[:, :, 0:2 * EL])
        for tq in range(1, 4):
            ts_ = slice(tq * (N // 4), (tq + 1) * (N // 4))
            nc.sync.dma_start(xt_sb[:, :, ts_], xt_dv[:, :, ts_])
        nc.sync.dma_start(wot_sb[:], wot_d.rearrange("p (pr j) -> p pr j", pr=NPAIRS))
        nc.sync.dma_start(bias_sb[:], bias_d)
        nc.vector.memset(vo_sb[:], 1.0)
        bqk_sb = bias_sb[:, 0:2 * NPAIRS]
        bv_bc = bias_sb[:, 2 * NPAIRS:]

        # "touch" ops: advance each engine's vector clock past the input
        # DMAs one semaphore at a time (keeps most instructions at a single
        # sync-wait so Bacc rarely needs to split waits into extra nops).
        tch_sb = singles.tile([NP, 16], FP32)
        nc.vector.tensor_copy(tch_sb[:, 0:8], bias_sb[:, 0:8])
        tch_ps = proj_ps.tile([NP, TC], FP32, tag="ps")
        nc.tensor.matmul(tch_ps[0:1, 0:1], lhsT=xt_sb[:, 0, 0:1],
                         rhs=xt_sb[:, 0, 0:1], start=True, stop=True)
        nc.tensor.matmul(tch_ps[0:1, 1:2], lhsT=wt_sb[:, 0, 0:1],
                         rhs=wt_sb[:, 0, 0:1], start=True, stop=True)
        nc.tensor.matmul(tch_ps[0:1, 2:3], lhsT=wot_sb[:, 0, 0:1],
                         rhs=wot_sb[:, 0, 0:1], start=True, stop=True)

        # ---- projections ----------------------------------------------------
        # V first: its DVE bias-add tick is then subsumed by the later Q/K
        # ticks, keeping the attention matmuls at a single sync-wait.
        for tb in range(NKB):
            ps = proj_ps.tile([NP, EL], FP32, tag="ps")
            for ec in range(EC):
                nc.tensor.matmul(
                    ps[:],
                    lhsT=xt_sb[:, ec, tb * NP:(tb + 1) * NP],
                    rhs=wt_sb[:, ec, 2 * EL:3 * EL],
                    start=(ec == 0),
                    stop=(ec == EC - 1),
                )
            psv = ps[:].rearrange("p (h two d) -> p h two d", two=2, d=HD)
            bvv = bv_bc.rearrange("p (h two d) -> p h two d", two=2, d=HD)
            vov = vo_sb[:, tb].rearrange("p (h two) f -> p h two f", two=2)
            # even heads of each pair -> cols 0:64, odd heads -> cols 64:128
            nc.vector.tensor_tensor(
                vov[:, :, 0, 0:HD], psv[:, :, 0, :], bvv[:, :, 0, :],
                mybir.AluOpType.add,
            )
            nc.vector.tensor_tensor(
                vov[:, :, 1, HD:NP], psv[:, :, 1, :], bvv[:, :, 1, :],
                mybir.AluOpType.add,
            )

        def qk_proj_jobs(p):
            """8 deferred jobs, each computing one [128, 512] Q^T/K^T tile."""
            for dst, coff, bcol in (
                (qt_sb, p * NP, p),
                (kt_sb, EL + p * NP, NPAIRS + p),
            ):
                for t in range(N // TC):
                    def job(dst=dst, coff=coff, bcol=bcol, t=t, p=p):
                        ps = proj_ps.tile([NP, TC], FP32, tag="ps")
                        for ec in range(EC):
                            nc.tensor.matmul(
                                ps[:],
                                lhsT=wt_sb[:, ec, coff:coff + NP],
                                rhs=xt_sb[:, ec, t * TC:(t + 1) * TC],
                                start=(ec == 0),
                                stop=(ec == EC - 1),
                            )
                        nc.vector.tensor_tensor(
                            dst[:, p, t * TC:(t + 1) * TC], ps[:],
                            bqk_sb[:, bcol:bcol + 1].to_broadcast((NP, TC)),
                            mybir.AluOpType.add,
                        )
                    yield job

        def outproj_jobs(q):
            """8 deferred jobs: out-projection for q-chunk's 4 token blocks.

            Staged into one SBUF buffer; the last job also issues the DMA.
            """
            yb = ypool.tile([NP, NQC, E], FP32, tag="yb")
            njobs = (QC // NP) * (E // TC)
            done = [0]
            for i, tb in enumerate(range(q * QC // NP, (q + 1) * QC // NP)):
                for jc in range(E // TC):
                    def job(i=i, tb=tb, jc=jc):
                        ps = proj_ps.tile([NP, TC], FP32, tag="ps")
                        for p in range(NPAIRS):
                            nc.tensor.matmul(
                                ps[:],
                                lhsT=at_sb[:, p, tb * NP:(tb + 1) * NP],
                                rhs=wot_sb[:, p, jc * TC:(jc + 1) * TC],
                                start=(p == 0),
                                stop=(p == NPAIRS - 1),
                            )
                        nc.vector.tensor_copy(
                            yb[:, i, jc * TC:(jc + 1) * TC], ps[:])
                        done[0] += 1
                        if done[0] == njobs:
                            nc.sync.dma_start(
                                y_d[:, q * NQC:(q + 1) * NQC, :], yb[:])
                    yield job

        # Q^T/K^T for pair 0 up-front; later pairs' projections and each
        # q-chunk's out-projection are drip-fed between attention groups so
        # PE work stays level with the ACT exp stream.
        for job in qk_proj_jobs(0):
            job()

        # ---- attention, pair-outer, with interleaved deferred work ---------
        for p in range(NPAIRS):
            if p < NPAIRS - 1:
                filler = qk_proj_jobs(p + 1)
                interval = 4          # 8 jobs over this pair's 32 groups
            else:
                filler = iter(())
                interval = 1
            gidx = 0
            for q in range(NQC):
                if p == NPAIRS - 1 and q > 0:
                    filler = outproj_jobs(q - 1)   # 8 jobs over 8 groups
                qs = slice(q * QC, (q + 1) * QC)
                pvA = pv_ps.tile([NP, QC], FP32, tag="pv")
                pvB = pv2_ps.tile([NP, QC], FP32, tag="pv2")
                for g2 in range(NKB // 2):
                    st0 = s_ps.tile([NP, 2, QC], FP32, tag="st")
                    st1 = s_ps.tile([NP, 2, QC], FP32, tag="st")
                    et0 = epool.tile([NP, 2, QC], BF16, tag="et")
                    et1 = epool.tile([NP, 2, QC], BF16, tag="et")
                    for j, st in ((0, st0), (1, st1)):
                        kb = 2 * g2 + j
                        ks = slice(kb * NP, (kb + 1) * NP)
                        # both heads of one key block in one tile; the two
                        # matmuls land on disjoint PE row groups and their
                        # staging slot frees as a unit -> they issue
                        # back-to-back and overlap in the array
                        nc.tensor.matmul(
                            st[:, 0, :],
                            lhsT=kt_sb[0:HD, p, ks], rhs=qt_sb[0:HD, p, qs],
                            start=True, stop=True,
                        )
                        nc.tensor.matmul(
                            st[:, 1, :],
                            lhsT=kt_sb[HD:NP, p, ks], rhs=qt_sb[HD:NP, p, qs],
                            start=True, stop=True,
                        )
                    # exp with the 1/sqrt(HD) score scale fused in
                    nc.scalar.activation(
                        et0[:], st0[:], mybir.ActivationFunctionType.Exp,
                        scale=0.125,
                    )
                    nc.scalar.activation(
                        et1[:], st1[:], mybir.ActivationFunctionType.Exp,
                        scale=0.125,
                    )
                    for j, et in ((0, et0), (1, et1)):
                        kb = 2 * g2 + j
                        first, last = (kb == 0), (kb == NKB - 1)
                        # fused PV+rowsum: full 128-col stationary operand
                        # pvA = [dataA | sumsA], pvB = [sumsB | dataB]
                        nc.tensor.matmul(
                            pvA[:],
                            lhsT=vo_sb[:, kb, 2 * p, :],
                            rhs=et[:, 0, :], start=first, stop=last,
                        )
                        nc.tensor.matmul(
                            pvB[:],
                            lhsT=vo_sb[:, kb, 2 * p + 1, :],
                            rhs=et[:, 1, :], start=first, stop=last,
                        )
                    gidx += 1
                    if gidx % interval == 0:
                        job = next(filler, None)
                        if job is not None:
                            job()
                # evacuate both PV banks to SBUF immediately so the next
                # chunk's matmuls can reuse them; the normalize chain below
                # then runs entirely off the PE critical path
                cA = rpool.tile([NP, QC], FP32, tag="cA")
                cB = rpool.tile([NP, QC], FP32, tag="cB")
                nc.vector.tensor_copy(cA[:], pvA[:])
                nc.vector.tensor_copy(cB[:], pvB[:])
                rcA = rpool.tile([NP, QC], FP32, tag="rcA")
                rcB = rpool.tile([NP, QC], FP32, tag="rcB")
                rc2 = rpool.tile([NP, QC], FP32, tag="rc2")
                # full-tile reciprocals (the unused data halves produce junk
                # that is never read); custom DVE ops run at partition base 0
                nc.vector.reciprocal_approx_fast(rcA[:], cA[:])
                nc.vector.reciprocal_approx_fast(rcB[:], cB[:])
                # move each head's 1/sum onto its data partitions
                nc.sync.dma_start(rc2[0:HD, :], rcA[HD:NP, :])
                nc.sync.dma_start(rc2[HD:NP, :], rcB[0:HD, :])
                nc.vector.tensor_mul(at_sb[0:HD, p, qs], cA[0:HD, :],
                                     rc2[0:HD, :])
                nc.vector.tensor_mul(at_sb[HD:NP, p, qs], cB[HD:NP, :],
                                     rc2[HD:NP, :])

        # final q-chunk's out-projection
        for job in outproj_jobs(NQC - 1):
            job()


def shard_inputs(qkv, in_proj_w, in_proj_b, out_proj_w):
    """Build the 8 per-core input maps (host-side transpose + bf16 cast).

    All device tensors are partition-major [128, free] so each DMA run is
    long and contiguous.
    """
    bf = ml_dtypes.bfloat16
    in_maps = []
    for c in range(8):
        b, g = c // 2, c % 2
        cs = slice(g * EL, (g + 1) * EL)
        # X^T [E, N] -> [p, ec*t]
        xt = np.ascontiguousarray(
            qkv[b].T.reshape(EC, NP, N).transpose(1, 0, 2).reshape(NP, EC * N)
        ).astype(bf)
        w_l = np.concatenate(
            [in_proj_w[cs], in_proj_w[E:2 * E][cs], in_proj_w[2 * E:3 * E][cs]], 0
        )  # [3*EL, E]
        wt = np.ascontiguousarray(
            w_l.T.reshape(EC, NP, 3 * EL).transpose(1, 0, 2).reshape(NP, -1)
        ).astype(bf)
        wot = np.ascontiguousarray(
            out_proj_w[:, cs].T.reshape(NPAIRS, NP, E).transpose(1, 0, 2)
            .reshape(NP, -1)
        ).astype(bf)
        bias = np.empty((NP, 2 * NPAIRS + EL), np.float32)
        bq = in_proj_b[cs]
        bk = in_proj_b[E:2 * E][cs]
        for p in range(NPAIRS):
            bias[:, p] = bq[p * NP:(p + 1) * NP]
            bias[:, NPAIRS + p] = bk[p * NP:(p + 1) * NP]
        bias[:, 2 * NPAIRS:] = in_proj_b[2 * E:3 * E][cs][None, :]
        in_maps.append({"xt": xt, "wt": wt, "wot": wot, "bias": bias})
    return in_maps


def unshard_output(ys, out_proj_b):
    # ys[c] is [128, 16, 1024] partition-major: y[tb*128+p, j] = ys[p, tb, j]
    full = [np.asarray(y).transpose(1, 0, 2).reshape(N, E) for y in ys]
    out = np.stack([full[2 * b] + full[2 * b + 1] for b in range(B)])
    out += out_proj_b[None, None, :]
    return out.astype(np.float32)


def kernel(qkv, in_proj_w, in_proj_b, out_proj_w, out_proj_b):
    qkv = np.asarray(qkv, np.float32)
    in_proj_w = np.asarray(in_proj_w, np.float32)
    in_proj_b = np.asarray(in_proj_b, np.float32)
    out_proj_w = np.asarray(out_proj_w, np.float32)
    out_proj_b = np.asarray(out_proj_b, np.float32)

    if "nc" not in _CACHED:
        _CACHED["nc"] = build_kernel()
    nc = _CACHED["nc"]

    in_maps = shard_inputs(qkv, in_proj_w, in_proj_b, out_proj_w)
    res = run_bass_kernel_spmd(nc, in_maps, core_ids=list(range(8)))
    ys = [res.results[c]["y"] for c in range(8)]
    return unshard_output(ys, out_proj_b)


# revision 17
# speedup vs baseline: 1.1211x; 1.1211x over previous
"""Trainium2 Bass kernel for CustomMultiHeadSelfAttention.

Problem shapes: B=4, N=2048, E=1024, H=16, HD=64.

Sharding (8 cores): core c -> batch b = c//2, head-group g = c%2
(heads 8g..8g+7, i.e. feature cols [512g, 512g+512) of q/k/v).
Each core:
  - projects its batch's qkv slice -> Q^T,K^T (head-pair packed, d on
    partitions) and V (natural, seq on partitions)
  - full attention for its 8 heads (exact softmax, no max subtraction --
    scores are ~N(0,1) so exp never overflows)
  - partial out_proj: attnout_local [2048,512] @ W_out[:,cols]^T -> [2048,1024]
Host unshards: y[b] = partial[2b] + partial[2b+1] + out_proj_b.

All matmuls run in bf16 with fp32 PSUM accumulation (host pre-casts the
sharded operands); softmax statistics stay fp32.

PE utilization tricks:
  - S^T: two heads as concurrent 64-row PE tiles (row groups 0-1 / 2-3)
  - PV:  two heads as concurrent 64-col PE tiles, outputs stacked into the
    [128 = 2*64] partition layout the out-projection wants
  - softmax row-sums: ones[128,64] matmul -> sums pre-broadcast across 64
    partitions, accumulated in PSUM alongside PV

DMA: all DRAM layouts are partition-major ([128, free]) so every transfer
is long contiguous runs per partition -- descriptor-dispatch on the
sequencers was the original bottleneck.
"""

import sys

if "/opt/trn_rl_repo" not in sys.path:
    sys.path.insert(0, "/opt/trn_rl_repo")

from contextlib import ExitStack

import ml_dtypes
import numpy as np

import concourse.tile as tile
from concourse import bacc, mybir
from concourse.bass_utils import run_bass_kernel_spmd

B, N, E, H = 4, 2048, 1024, 16
HD = E // H          # 64
HL = H // 2          # 8 local heads per core
EL = HL * HD         # 512 local feature cols per core
NP = 128             # partitions
NPAIRS = HL // 2     # 4 head pairs per core (2 heads packed in 128 partitions)
QC = 512             # query chunk (free dim of S^T / PV matmuls)
NQC = N // QC        # 4
NKB = N // NP        # 16 key blocks of 128
TC = 512             # token chunk in projections
EC = E // NP         # 8 contraction chunks in the in-projection

BF16 = mybir.dt.bfloat16
FP32 = mybir.dt.float32

_CACHED = {}


def build_kernel():
    """Build the per-core Bass program (identical for every core)."""
    nc = bacc.Bacc("TRN2", target_bir_lowering=False, debug=False, num_devices=8)

    # bf16 operand blobs + one fp32 bias blob, all partition-major
    xt_d = nc.dram_tensor("xt", [NP, EC * N], BF16, kind="ExternalInput").ap()
    wt_d = nc.dram_tensor("wt", [NP, EC * 3 * EL], BF16, kind="ExternalInput").ap()
    wot_d = nc.dram_tensor("wot", [NP, NPAIRS * E], BF16, kind="ExternalInput").ap()
    bias_d = nc.dram_tensor(
        "bias", [NP, 2 * NPAIRS + EL], FP32, kind="ExternalInput"
    ).ap()
    # output, partition-major: y_d[p, tb, j] = y[tb*128 + p, j]
    y_d = nc.dram_tensor("y", [NP, NKB, E], FP32, kind="ExternalOutput").ap()

    with tile.TileContext(nc) as tc:
        _emit(tc, xt_d, wt_d, wot_d, bias_d, y_d)
    nc.compile()
    return nc


def _emit(tc, xt_d, wt_d, wot_d, bias_d, y_d):
    nc = tc.nc
    ctx = ExitStack()
    with ctx:
        singles = ctx.enter_context(tc.tile_pool(name="singles", bufs=1))
        proj_ps = ctx.enter_context(tc.tile_pool(name="proj_ps", bufs=2, space="PSUM"))
        s_ps = ctx.enter_context(tc.tile_pool(name="s_ps", bufs=2, space="PSUM"))
        pv_ps = ctx.enter_context(tc.tile_pool(name="pv_ps", bufs=1, space="PSUM"))
        pv2_ps = ctx.enter_context(tc.tile_pool(name="pv2_ps", bufs=1, space="PSUM"))
        epool = ctx.enter_context(tc.tile_pool(name="epool", bufs=4))
        rpool = ctx.enter_context(tc.tile_pool(name="rpool", bufs=2))
        dpool = ctx.enter_context(tc.tile_pool(name="dpool", bufs=2, space="DRAM"))
        ypool = ctx.enter_context(tc.tile_pool(name="ypool", bufs=1))

        # ---- resident SBUF tensors -----------------------------------------
        xt_sb = singles.tile([NP, EC, N], BF16)           # X^T  [e, t]
        wt_sb = singles.tile([NP, EC, 3 * EL], BF16)      # W_in^T [e, c]
        wot_sb = singles.tile([NP, NPAIRS, E], BF16)      # W_out^T [el, j]
        qt_sb = singles.tile([NP, NPAIRS, N], BF16)       # Q^T (pair-packed)
        kt_sb = singles.tile([NP, NPAIRS, N], BF16)       # K^T (pair-packed)
        # V packed per head as a 128-col stationary block: even head in a
        # pair -> [V_h | ones], odd head -> [ones | V_h].  The ones block
        # makes every PV matmul also produce that head's softmax row-sums,
        # broadcast across 64 partitions, on the half not holding data.
        vo_sb = singles.tile([NP, NKB, HL, NP], BF16)
        at_sb = singles.tile([NP, NPAIRS, N], BF16)       # attnout^T (pair-packed)
        bias_sb = singles.tile([NP, 2 * NPAIRS + EL], FP32)

        xt_dv = xt_d.rearrange("p (ec t) -> p ec t", ec=EC)
        wt_dv = wt_d.rearrange("p (ec c) -> p ec c", ec=EC)
        # V-weights first, then Q/K weights; X^T in quarters -- so the first
        # V-projection matmul isn't gated on the whole 8.4 MB input load
        nc.sync.dma_start(wt_sb[:, :, 2 * EL:3 * EL], wt_dv[:, :, 2 * EL:3 * EL])
        nc.sync.dma_start(xt_sb[:, :, 0:N // 2], xt_dv[:, :, 0:N // 2])
        nc.sync.dma_start(wt_sb[:, :, 0:2 * EL], wt_dv[:, :, 0:2 * EL])
        nc.sync.dma_start(xt_sb[:, :, N // 2:], xt_dv[:, :, N // 2:])
        nc.sync.dma_start(wot_sb[:], wot_d.rearrange("p (pr j) -> p pr j", pr=NPAIRS))
        nc.sync.dma_start(bias_sb[:], bias_d)
        nc.vector.memset(vo_sb[:], 1.0)
        bqk_sb = bias_sb[:, 0:2 * NPAIRS]
        bv_bc = bias_sb[:, 2 * NPAIRS:]

        # "touch" ops: advance each engine's vector clock past the input
        # DMAs one semaphore at a time (keeps most instructions at a single
        # sync-wait so Bacc rarely needs to split waits into extra nops).
        tch_sb = singles.tile([NP, 16], FP32)
        nc.vector.tensor_copy(tch_sb[:, 0:8], bias_sb[:, 0:8])
        tch_ps = proj_ps.tile([NP, TC], FP32, tag="ps")
        nc.tensor.matmul(tch_ps[0:1, 0:1], lhsT=xt_sb[:, 0, 0:1],
                         rhs=xt_sb[:, 0, 0:1], start=True, stop=True)
        nc.tensor.matmul(tch_ps[0:1, 1:2], lhsT=wt_sb[:, 0, 0:1],
                         rhs=wt_sb[:, 0, 0:1], start=True, stop=True)
        nc.tensor.matmul(tch_ps[0:1, 2:3], lhsT=wot_sb[:, 0, 0:1],
                         rhs=wot_sb[:, 0, 0:1], start=True, stop=True)

        # ---- projections ----------------------------------------------------
        # V first: its DVE bias-add tick is then subsumed by the later Q/K
        # ticks, keeping the attention matmuls at a single sync-wait.
        for tb in range(NKB):
            ps = proj_ps.tile([NP, EL], FP32, tag="ps")
            for ec in range(EC):
                nc.tensor.matmul(
                    ps[:],
                    lhsT=xt_sb[:, ec, tb * NP:(tb + 1) * NP],
                    rhs=wt_sb[:, ec, 2 * EL:3 * EL],
                    start=(ec == 0),
                    stop=(ec == EC - 1),
                )
            psv = ps[:].rearrange("p (h two d) -> p h two d", two=2, d=HD)
            bvv = bv_bc.rearrange("p (h two d) -> p h two d", two=2, d=HD)
            vov = vo_sb[:, tb].rearrange("p (h two) f -> p h two f", two=2)
            # even heads of each pair -> cols 0:64, odd heads -> cols 64:128
            nc.vector.tensor_tensor(
                vov[:, :, 0, 0:HD], psv[:, :, 0, :], bvv[:, :, 0, :],
                mybir.AluOpType.add,
            )
            nc.vector.tensor_tensor(
                vov[:, :, 1, HD:NP], psv[:, :, 1, :], bvv[:, :, 1, :],
                mybir.AluOpType.add,
            )

        def qk_proj_jobs(p):
            """8 deferred jobs, each computing one [128, 512] Q^T/K^T tile."""
            for dst, coff, bcol in (
                (qt_sb, p * NP, p),
                (kt_sb, EL + p * NP, NPAIRS + p),
            ):
                for t in range(N // TC):
                    def job(dst=dst, coff=coff, bcol=bcol, t=t, p=p):
                        ps = proj_ps.tile([NP, TC], FP32, tag="ps")
                        for ec in range(EC):
                            nc.tensor.matmul(
                                ps[:],
                                lhsT=wt_sb[:, ec, coff:coff + NP],
                                rhs=xt_sb[:, ec, t * TC:(t + 1) * TC],
                                start=(ec == 0),
                                stop=(ec == EC - 1),
                            )
                        nc.vector.tensor_tensor(
                            dst[:, p, t * TC:(t + 1) * TC], ps[:],
                            bqk_sb[:, bcol:bcol + 1].to_broadcast((NP, TC)),
                            mybir.AluOpType.add,
                        )
                    yield job

        def outproj_jobs(q):
            """8 deferred jobs: out-projection for q-chunk's 4 token blocks.

            Staged into one SBUF buffer; the last job also issues the DMA.
            """
            yb = ypool.tile([NP, NQC, E], FP32, tag="yb")
            njobs = (QC // NP) * (E // TC)
            done = [0]
            for i, tb in enumerate(range(q * QC // NP, (q + 1) * QC // NP)):
                for jc in range(E // TC):
                    def job(i=i, tb=tb, jc=jc):
                        ps = proj_ps.tile([NP, TC], FP32, tag="ps")
                        for p in range(NPAIRS):
                            nc.tensor.matmul(
                                ps[:],
                                lhsT=at_sb[:, p, tb * NP:(tb + 1) * NP],
                                rhs=wot_sb[:, p, jc * TC:(jc + 1) * TC],
                                start=(p == 0),
                                stop=(p == NPAIRS - 1),
                            )
                        nc.vector.tensor_copy(
                            yb[:, i, jc * TC:(jc + 1) * TC], ps[:])
                        done[0] += 1
                        if done[0] == njobs:
                            nc.sync.dma_start(
                                y_d[:, q * NQC:(q + 1) * NQC, :], yb[:])
                    yield job

        # Q^T/K^T for pair 0 up-front; later pairs' projections and each
        # q-chunk's out-projection are drip-fed between attention groups so
        # PE work stays level with the ACT exp stream.
        for job in qk_proj_jobs(0):
            job()

        # ---- attention with interleaved deferred work ----------------------
        for q in range(NQC):
            qs = slice(q * QC, (q + 1) * QC)
            for p in range(NPAIRS):
                if q == 0 and p < NPAIRS - 1:
                    filler = qk_proj_jobs(p + 1)
                elif q > 0 and p == 0:
                    filler = outproj_jobs(q - 1)
                else:
                    filler = iter(())
                pvA = pv_ps.tile([NP, QC], FP32, tag="pv")
                pvB = pv2_ps.tile([NP, QC], FP32, tag="pv2")
                for g2 in range(NKB // 2):
                    st0 = s_ps.tile([NP, 2, QC], FP32, tag="st")
                    st1 = s_ps.tile([NP, 2, QC], FP32, tag="st")
                    et0 = epool.tile([NP, 2, QC], BF16, tag="et")
                    et1 = epool.tile([NP, 2, QC], BF16, tag="et")
                    for j, st in ((0, st0), (1, st1)):
                        kb = 2 * g2 + j
                        ks = slice(kb * NP, (kb + 1) * NP)
                        # both heads of one key block in one tile; the two
                        # matmuls land on disjoint PE row groups and their
                        # staging slot frees as a unit -> they issue
                        # back-to-back and overlap in the array
                        nc.tensor.matmul(
                            st[:, 0, :],
                            lhsT=kt_sb[0:HD, p, ks], rhs=qt_sb[0:HD, p, qs],
                            start=True, stop=True,
                        )
                        nc.tensor.matmul(
                            st[:, 1, :],
                            lhsT=kt_sb[HD:NP, p, ks], rhs=qt_sb[HD:NP, p, qs],
                            start=True, stop=True,
                        )
                    # exp with the 1/sqrt(HD) score scale fused in
                    nc.scalar.activation(
                        et0[:], st0[:], mybir.ActivationFunctionType.Exp,
                        scale=0.125,
                    )
                    nc.scalar.activation(
                        et1[:], st1[:], mybir.ActivationFunctionType.Exp,
                        scale=0.125,
                    )
                    for j, et in ((0, et0), (1, et1)):
                        kb = 2 * g2 + j
                        first, last = (kb == 0), (kb == NKB - 1)
                        # fused PV+rowsum: full 128-col stationary operand
                        # pvA = [dataA | sumsA], pvB = [sumsB | dataB]
                        nc.tensor.matmul(
                            pvA[:],
                            lhsT=vo_sb[:, kb, 2 * p, :],
                            rhs=et[:, 0, :], start=first, stop=last,
                        )
                        nc.tensor.matmul(
                            pvB[:],
                            lhsT=vo_sb[:, kb, 2 * p + 1, :],
                            rhs=et[:, 1, :], start=first, stop=last,
                        )
                    job = next(filler, None)
                    if job is not None:
                        job()
                # evacuate both PV banks to SBUF immediately so the next
                # pair's matmuls can reuse them; the normalize chain below
                # then runs entirely off the PE critical path
                cA = rpool.tile([NP, QC], FP32, tag="cA")
                cB = rpool.tile([NP, QC], FP32, tag="cB")
                nc.vector.tensor_copy(cA[:], pvA[:])
                nc.vector.tensor_copy(cB[:], pvB[:])
                rcA = rpool.tile([NP, QC], FP32, tag="rcA")
                rcB = rpool.tile([NP, QC], FP32, tag="rcB")
                rc2 = rpool.tile([NP, QC], FP32, tag="rc2")
                # full-tile reciprocals (the unused data halves produce junk
                # that is never read); custom DVE ops run at partition base 0
                nc.vector.reciprocal_approx_fast(rcA[:], cA[:])
                nc.vector.reciprocal_approx_fast(rcB[:], cB[:])
                # move each head's 1/sum onto its data partitions
                nc.sync.dma_start(rc2[0:HD, :], rcA[HD:NP, :])
                nc.sync.dma_start(rc2[HD:NP, :], rcB[0:HD, :])
                nc.vector.tensor_mul(at_sb[0:HD, p, qs], cA[0:HD, :],
                                     rc2[0:HD, :])
                nc.vector.tensor_mul(at_sb[HD:NP, p, qs], cB[HD:NP, :],
                                     rc2[HD:NP, :])

        # final q-chunk's out-projection
        for job in outproj_jobs(NQC - 1):
            job()


def shard_inputs(qkv, in_proj_w, in_proj_b, out_proj_w):
    """Build the 8 per-core input maps (host-side transpose + bf16 cast).

    All device tensors are partition-major [128, free] so each DMA run is
    long and contiguous.
    """
    bf = ml_dtypes.bfloat16
    in_maps = []
    for c in range(8):
        b, g = c // 2, c % 2
        cs = slice(g * EL, (g + 1) * EL)
        # X^T [E, N] -> [p, ec*t]
        xt = np.ascontiguousarray(
            qkv[b].T.reshape(EC, NP, N).transpose(1, 0, 2).reshape(NP, EC * N)
        ).astype(bf)
        w_l = np.concatenate(
            [in_proj_w[cs], in_proj_w[E:2 * E][cs], in_proj_w[2 * E:3 * E][cs]], 0
        )  # [3*EL, E]
        wt = np.ascontiguousarray(
            w_l.T.reshape(EC, NP, 3 * EL).transpose(1, 0, 2).reshape(NP, -1)
        ).astype(bf)
        wot = np.ascontiguousarray(
            out_proj_w[:, cs].T.reshape(NPAIRS, NP, E).transpose(1, 0, 2)
            .reshape(NP, -1)
        ).astype(bf)
        bias = np.empty((NP, 2 * NPAIRS + EL), np.float32)
        bq = in_proj_b[cs]
        bk = in_proj_b[E:2 * E][cs]
        for p in range(NPAIRS):
            bias[:, p] = bq[p * NP:(p + 1) * NP]
            bias[:, NPAIRS + p] = bk[p * NP:(p + 1) * NP]
        bias[:, 2 * NPAIRS:] = in_proj_b[2 * E:3 * E][cs][None, :]
        in_maps.append({"xt": xt, "wt": wt, "wot": wot, "bias": bias})
    return in_maps


def unshard_output(ys, out_proj_b):
    # ys[c] is [128, 16, 1024] partition-major: y[tb*128+p, j] = ys[p, tb, j]
    full = [np.asarray(y).transpose(1, 0, 2).reshape(N, E) for y in ys]
    out = np.stack([full[2 * b] + full[2 * b + 1] for b in range(B)])
    out += out_proj_b[None, None, :]
    return out.astype(np.float32)


def kernel(qkv, in_proj_w, in_proj_b, out_proj_w, out_proj_b):
    qkv = np.asarray(qkv, np.float32)
    in_proj_w = np.asarray(in_proj_w, np.float32)
    in_proj_b = np.asarray(in_proj_b, np.float32)
    out_proj_w = np.asarray(out_proj_w, np.float32)
    out_proj_b = np.asarray(out_proj_b, np.float32)

    if "nc" not in _CACHED:
        _CACHED["nc"] = build_kernel()
    nc = _CACHED["nc"]

    in_maps = shard_inputs(qkv, in_proj_w, in_proj_b, out_proj_w)
    res = run_bass_kernel_spmd(nc, in_maps, core_ids=list(range(8)))
    ys = [res.results[c]["y"] for c in range(8)]
    return unshard_output(ys, out_proj_b)
